# revision 2
# baseline (speedup 1.0000x reference)
"""Trainium2 Bass kernel for quantized KV-cache dequant + scatter-update.

Reference semantics (per full tensors):
    k = k_cache_q.astype(f32) * k_scales + k_zero_points        # [B,H,L,D]
    v = v_cache_q.astype(f32) * v_scales + v_zero_points
    k = k.at[:, :, input_pos, :].set(k_val)
    v = v.at[:, :, input_pos, :].set(v_val)
    mask_out = mask.at[:, :, :, input_pos].set(True)
    return stack([k, v]), mask_out

Sharding: 8 NeuronCores, split on n_heads (H=32 -> 4 heads/core). Each core
runs an identical Bass program over its [B * H_local = 8] (b,h) slices.

Per-core device program (fast path, input_pos == arange(S)):
  - dequant: load cache slice as [128 partitions = L-row groups, free = 64*D],
    DVE tensor_mul by broadcast scale + in-place tensor_add of broadcast zp
    (int32 -> f32 conversion happens inside the DVE read), store.
  - rows [0:S) of the output are never dequant-stored (they are partitions
    0..S/64-1 in this layout); instead k_val/v_val are copied DRAM->DRAM.
  - mask: copy through SBUF with a memset of the [0:S) byte range per slice.
"""

import sys

for _p in ("/opt/trn_rl_repo", "/root/.axon_site/_ro/trn_rl_repo"):
    if _p not in sys.path:
        sys.path.append(_p)

import numpy as np

B, H, L, D = 2, 32, 8192, 128
S = 256
N_CORES = 8
H_LOC = H // N_CORES          # heads per core
SL = B * H_LOC                # (b, h) slices per core per cache
P = 128                       # SBUF partitions
FLAT = L * D // P             # free elems per partition per slice (8192)
CH = 4096                     # free elems per chunk (2 chunks per slice)
NCH = FLAT // CH
P0 = S * D // FLAT            # partitions fully covered by the scatter rows (4)

_CACHE: dict = {}


def _build_program(fast: bool):
    """Build + compile the per-core Bass program. fast=True assumes the
    scatter rows are exactly [0:S) (so dequant skips partitions < P0 and the
    scatter/mask update runs on device)."""
    import concourse.bacc as bacc
    import concourse.mybir as mybir
    from concourse.tile import TileContext

    dt = mybir.dt
    nc = bacc.Bacc("TRN2", target_bir_lowering=False, debug=False,
                   num_devices=N_CORES)

    kq = nc.declare_dram_parameter("kq", [SL, L, D], dt.int32, isOutput=False)
    vq = nc.declare_dram_parameter("vq", [SL, L, D], dt.int32, isOutput=False)
    ks = nc.declare_dram_parameter("ks", [SL, D], dt.float32, isOutput=False)
    kz = nc.declare_dram_parameter("kz", [SL, D], dt.float32, isOutput=False)
    vs = nc.declare_dram_parameter("vs", [SL, D], dt.float32, isOutput=False)
    vz = nc.declare_dram_parameter("vz", [SL, D], dt.float32, isOutput=False)
    kval = nc.declare_dram_parameter("kval", [SL, S, D], dt.float32, isOutput=False)
    vval = nc.declare_dram_parameter("vval", [SL, S, D], dt.float32, isOutput=False)
    mk = nc.declare_dram_parameter("mk", [SL, L], dt.uint8, isOutput=False)
    ko = nc.declare_dram_parameter("ko", [SL, L, D], dt.float32, isOutput=True)
    vo = nc.declare_dram_parameter("vo", [SL, L, D], dt.float32, isOutput=True)
    mo = nc.declare_dram_parameter("mo", [SL, L], dt.uint8, isOutput=True)

    p_lo = P0 if fast else 0
    mrow = SL * L // P  # mask free bytes per partition (512)

    with TileContext(nc) as tc:
        with tc.tile_pool(name="const", bufs=1) as cpool, \
             tc.tile_pool(name="io", bufs=3) as io:
            # Broadcast all per-(slice, d) scales/zps to every partition:
            # [1, SL*D] -> [128, SL*D].
            bcast = {}
            for name, src in (("ks", ks), ("kz", kz), ("vs", vs), ("vz", vz)):
                t = cpool.tile([P, SL * D], dt.float32, tag=name)
                nc.sync.dma_start(
                    out=t[:],
                    in_=src[:].rearrange("(o s) d -> o (s d)", o=1)
                              .broadcast_to([P, SL * D]),
                )
                bcast[name] = t

            # Mask: [SL, L] u8 through SBUF; memset the scatter range to True.
            mt = cpool.tile([SL, L], dt.uint8, tag="mt")
            nc.sync.dma_start(out=mt[:], in_=mk[:])
            if fast:
                nc.vector.memset(mt[0:SL, 0:S], 1)
            nc.scalar.dma_start(out=mo[:], in_=mt[:])

            if fast:
                # New K/V rows: DRAM -> DRAM, disjoint from dequant stores.
                nc.sync.dma_start(out=ko[:, 0:S, :], in_=kval[:])
                nc.sync.dma_start(out=vo[:, 0:S, :], in_=vval[:])

            for q_t, o_t, sb, zb in (
                (kq, ko, bcast["ks"], bcast["kz"]),
                (vq, vo, bcast["vs"], bcast["vz"]),
            ):
                for s in range(SL):
                    qf = q_t[s].rearrange("(p n) d -> p (n d)", p=P)
                    of = o_t[s].rearrange("(p n) d -> p (n d)", p=P)
                    s3 = (sb[:, s * D:(s + 1) * D]
                          .rearrange("p (n d) -> p n d", n=1)
                          .broadcast_to([P, CH // D, D]))
                    z3 = (zb[:, s * D:(s + 1) * D]
                          .rearrange("p (n d) -> p n d", n=1)
                          .broadcast_to([P, CH // D, D]))
                    for c in range(NCH):
                        qt = io.tile([P, CH], dt.int32, tag="qin")
                        nc.sync.dma_start(
                            out=qt[p_lo:], in_=qf[p_lo:, c * CH:(c + 1) * CH])
                        ot = io.tile([P, CH], dt.float32, tag="out")
                        q3 = qt[:].rearrange("p (n d) -> p n d", d=D)
                        o3 = ot[:].rearrange("p (n d) -> p n d", d=D)
                        nc.vector.tensor_mul(out=o3, in0=q3, in1=s3)
                        nc.vector.tensor_add(out=o3, in0=o3, in1=z3)
                        nc.scalar.dma_start(
                            out=of[p_lo:, c * CH:(c + 1) * CH], in_=ot[p_lo:])

    nc.compile()
    return nc


def _get_program(fast: bool):
    key = ("prog", fast)
    if key not in _CACHE:
        _CACHE[key] = _build_program(fast)
    return _CACHE[key]


def _shard_inputs(k_cache_q, k_scales, k_zero_points, v_cache_q, v_scales,
                  v_zero_points, k_val, v_val, mask):
    """Slice full tensors into per-core input maps (head-parallel)."""
    in_maps = []
    for c in range(N_CORES):
        hs = slice(c * H_LOC, (c + 1) * H_LOC)
        cont = np.ascontiguousarray
        in_maps.append({
            "kq": cont(k_cache_q[:, hs]).reshape(SL, L, D),
            "vq": cont(v_cache_q[:, hs]).reshape(SL, L, D),
            "ks": cont(k_scales[:, hs]).reshape(SL, D),
            "kz": cont(k_zero_points[:, hs]).reshape(SL, D),
            "vs": cont(v_scales[:, hs]).reshape(SL, D),
            "vz": cont(v_zero_points[:, hs]).reshape(SL, D),
            "kval": cont(k_val[:, hs]).reshape(SL, S, D),
            "vval": cont(v_val[:, hs]).reshape(SL, S, D),
            "mk": cont(mask[:, hs]).reshape(SL, L).view(np.uint8),
        })
    return in_maps


def _assemble(results):
    k = np.concatenate(
        [results[c]["ko"].reshape(B, H_LOC, L, D) for c in range(N_CORES)],
        axis=1)
    v = np.concatenate(
        [results[c]["vo"].reshape(B, H_LOC, L, D) for c in range(N_CORES)],
        axis=1)
    m = np.concatenate(
        [results[c]["mo"].view(np.bool_).reshape(B, H_LOC, 1, L)
         for c in range(N_CORES)],
        axis=1)
    return k, v, m


def kernel(k_cache_q, k_scales, k_zero_points, v_cache_q, v_scales,
           v_zero_points, k_val, v_val, input_pos, mask):
    from concourse.bass_utils import run_bass_kernel_spmd

    k_cache_q = np.asarray(k_cache_q, dtype=np.int32)
    v_cache_q = np.asarray(v_cache_q, dtype=np.int32)
    k_scales = np.asarray(k_scales, dtype=np.float32)
    k_zero_points = np.asarray(k_zero_points, dtype=np.float32)
    v_scales = np.asarray(v_scales, dtype=np.float32)
    v_zero_points = np.asarray(v_zero_points, dtype=np.float32)
    k_val = np.asarray(k_val, dtype=np.float32)
    v_val = np.asarray(v_val, dtype=np.float32)
    input_pos = np.asarray(input_pos, dtype=np.int32)
    mask = np.asarray(mask, dtype=np.bool_)

    fast = input_pos.shape == (S,) and bool(
        (input_pos == np.arange(S, dtype=np.int32)).all())

    nc = _get_program(fast)
    in_maps = _shard_inputs(k_cache_q, k_scales, k_zero_points, v_cache_q,
                            v_scales, v_zero_points, k_val, v_val, mask)
    res = run_bass_kernel_spmd(nc, in_maps, core_ids=list(range(N_CORES)))
    k, v, m = _assemble(res.results)

    if not fast:
        # General-position fallback: apply the scatter on host.
        k[:, :, input_pos, :] = k_val
        v[:, :, input_pos, :] = v_val
        m[:, :, :, input_pos] = True

    return np.stack([k, v]), m


# revision 5
# speedup vs baseline: 1.8762x; 1.8762x over previous
"""Trainium2 Bass kernel for quantized KV-cache dequant + scatter-update.

Reference semantics (per full tensors):
    k = k_cache_q.astype(f32) * k_scales + k_zero_points        # [B,H,L,D]
    v = v_cache_q.astype(f32) * v_scales + v_zero_points
    k = k.at[:, :, input_pos, :].set(k_val)
    v = v.at[:, :, input_pos, :].set(v_val)
    mask_out = mask.at[:, :, :, input_pos].set(True)
    return stack([k, v]), mask_out

Sharding: 8 NeuronCores, split on n_heads (H=32 -> 4 heads/core). Each core
runs an identical Bass program over its [B * H_local = 8] (b,h) slices.

Per-core device program (fast path, input_pos == arange(S)):
  - dequant: load cache slice as [128 partitions = L-row groups, free = 64*D],
    DVE tensor_mul by broadcast scale + in-place tensor_add of broadcast zp
    (int32 -> f32 conversion happens inside the DVE read), store.
  - rows [0:S) of the output are never dequant-stored (they are partitions
    0..S/64-1 in this layout); instead k_val/v_val are copied DRAM->DRAM.
  - mask: copy through SBUF with a memset of the [0:S) byte range per slice.
"""

import sys

for _p in ("/opt/trn_rl_repo", "/root/.axon_site/_ro/trn_rl_repo"):
    if _p not in sys.path:
        sys.path.append(_p)

import numpy as np

B, H, L, D = 2, 32, 8192, 128
S = 256
N_CORES = 8
H_LOC = H // N_CORES          # heads per core
SL = B * H_LOC                # (b, h) slices per core per cache
P = 128                       # SBUF partitions
FLAT = L * D // P             # free elems per partition per slice (8192)
CH = 4096                     # free elems per chunk (2 chunks per slice)
NCH = FLAT // CH
P0 = S * D // FLAT            # partitions fully covered by the scatter rows (4)

_CACHE: dict = {}


def _build_program(fast: bool, reps: int = 1):
    """Build + compile the per-core Bass program. fast=True assumes the
    scatter rows are exactly [0:S) (so dequant skips partitions < P0 and the
    scatter/mask update runs on device). reps>1 repeats the whole body with
    all-engine barriers between repetitions (timing amortization only)."""
    import concourse.bacc as bacc
    import concourse.mybir as mybir
    from concourse.tile import TileContext

    dt = mybir.dt
    nc = bacc.Bacc("TRN2", target_bir_lowering=False, debug=False,
                   num_devices=N_CORES)

    kq = nc.declare_dram_parameter("kq", [SL, L, D], dt.int32, isOutput=False)
    vq = nc.declare_dram_parameter("vq", [SL, L, D], dt.int32, isOutput=False)
    ks = nc.declare_dram_parameter("ks", [SL, D], dt.float32, isOutput=False)
    kz = nc.declare_dram_parameter("kz", [SL, D], dt.float32, isOutput=False)
    vs = nc.declare_dram_parameter("vs", [SL, D], dt.float32, isOutput=False)
    vz = nc.declare_dram_parameter("vz", [SL, D], dt.float32, isOutput=False)
    kval = nc.declare_dram_parameter("kval", [SL, S, D], dt.float32, isOutput=False)
    vval = nc.declare_dram_parameter("vval", [SL, S, D], dt.float32, isOutput=False)
    mk = nc.declare_dram_parameter("mk", [SL, L], dt.uint8, isOutput=False)
    ko = nc.declare_dram_parameter("ko", [SL, L, D], dt.float32, isOutput=True)
    vo = nc.declare_dram_parameter("vo", [SL, L, D], dt.float32, isOutput=True)
    mo = nc.declare_dram_parameter("mo", [SL, L], dt.uint8, isOutput=True)

    p_lo = P0 if fast else 0
    mrow = SL * L // P  # mask free bytes per partition (512)

    with TileContext(nc) as tc:
        with tc.tile_pool(name="const", bufs=1) as cpool, \
             tc.tile_pool(name="io", bufs=3) as io:
          for rep in range(reps):
            if rep:
                tc.strict_bb_all_engine_barrier()
            # Broadcast all per-(slice, d) scales/zps to every partition:
            # [1, SL*D] -> [128, SL*D].
            bcast = {}
            for name, src in (("ks", ks), ("kz", kz), ("vs", vs), ("vz", vz)):
                t = cpool.tile([P, SL * D], dt.float32, tag=name)
                nc.sync.dma_start(
                    out=t[:],
                    in_=src[:].rearrange("(o s) d -> o (s d)", o=1)
                              .broadcast_to([P, SL * D]),
                )
                bcast[name] = t

            # Mask: [SL, L] u8 through SBUF; memset the scatter range to True.
            mt = cpool.tile([SL, L], dt.uint8, tag="mt")
            nc.sync.dma_start(out=mt[:], in_=mk[:])
            if fast:
                nc.vector.memset(mt[0:SL, 0:S], 1)
            nc.scalar.dma_start(out=mo[:], in_=mt[:])

            if fast:
                # New K/V rows: DRAM -> DRAM, disjoint from dequant stores.
                nc.sync.dma_start(out=ko[:, 0:S, :], in_=kval[:])
                nc.sync.dma_start(out=vo[:, 0:S, :], in_=vval[:])

            for q_t, o_t, sb, zb in (
                (kq, ko, bcast["ks"], bcast["kz"]),
                (vq, vo, bcast["vs"], bcast["vz"]),
            ):
                for s in range(SL):
                    qf = q_t[s].rearrange("(p n) d -> p (n d)", p=P)
                    of = o_t[s].rearrange("(p n) d -> p (n d)", p=P)
                    s3 = (sb[:, s * D:(s + 1) * D]
                          .rearrange("p (n d) -> p n d", n=1)
                          .broadcast_to([P, CH // D, D]))
                    z3 = (zb[:, s * D:(s + 1) * D]
                          .rearrange("p (n d) -> p n d", n=1)
                          .broadcast_to([P, CH // D, D]))
                    for c in range(NCH):
                        qt = io.tile([P, CH], dt.int32, tag="qin")
                        nc.sync.dma_start(
                            out=qt[p_lo:], in_=qf[p_lo:, c * CH:(c + 1) * CH])
                        ot = io.tile([P, CH], dt.float32, tag="out")
                        q3 = qt[:].rearrange("p (n d) -> p n d", d=D)
                        o3 = ot[:].rearrange("p (n d) -> p n d", d=D)
                        nc.vector.tensor_mul(out=o3, in0=q3, in1=s3)
                        nc.vector.tensor_add(out=o3, in0=o3, in1=z3)
                        nc.scalar.dma_start(
                            out=of[p_lo:, c * CH:(c + 1) * CH], in_=ot[p_lo:])

    nc.compile()
    return nc


def _get_program(fast: bool, reps: int = 1):
    key = ("prog", fast, reps)
    if key not in _CACHE:
        _CACHE[key] = _build_program(fast, reps)
    return _CACHE[key]


def _shard_inputs(k_cache_q, k_scales, k_zero_points, v_cache_q, v_scales,
                  v_zero_points, k_val, v_val, mask):
    """Slice full tensors into per-core input maps (head-parallel)."""
    in_maps = []
    for c in range(N_CORES):
        hs = slice(c * H_LOC, (c + 1) * H_LOC)
        cont = np.ascontiguousarray
        in_maps.append({
            "kq": cont(k_cache_q[:, hs]).reshape(SL, L, D),
            "vq": cont(v_cache_q[:, hs]).reshape(SL, L, D),
            "ks": cont(k_scales[:, hs]).reshape(SL, D),
            "kz": cont(k_zero_points[:, hs]).reshape(SL, D),
            "vs": cont(v_scales[:, hs]).reshape(SL, D),
            "vz": cont(v_zero_points[:, hs]).reshape(SL, D),
            "kval": cont(k_val[:, hs]).reshape(SL, S, D),
            "vval": cont(v_val[:, hs]).reshape(SL, S, D),
            "mk": cont(mask[:, hs]).reshape(SL, L).view(np.uint8),
        })
    return in_maps


def _assemble(results):
    k = np.concatenate(
        [results[c]["ko"].reshape(B, H_LOC, L, D) for c in range(N_CORES)],
        axis=1)
    v = np.concatenate(
        [results[c]["vo"].reshape(B, H_LOC, L, D) for c in range(N_CORES)],
        axis=1)
    m = np.concatenate(
        [results[c]["mo"].view(np.bool_).reshape(B, H_LOC, 1, L)
         for c in range(N_CORES)],
        axis=1)
    return k, v, m


def kernel(k_cache_q, k_scales, k_zero_points, v_cache_q, v_scales,
           v_zero_points, k_val, v_val, input_pos, mask):
    from concourse.bass_utils import run_bass_kernel_spmd

    k_cache_q = np.asarray(k_cache_q, dtype=np.int32)
    v_cache_q = np.asarray(v_cache_q, dtype=np.int32)
    k_scales = np.asarray(k_scales, dtype=np.float32)
    k_zero_points = np.asarray(k_zero_points, dtype=np.float32)
    v_scales = np.asarray(v_scales, dtype=np.float32)
    v_zero_points = np.asarray(v_zero_points, dtype=np.float32)
    k_val = np.asarray(k_val, dtype=np.float32)
    v_val = np.asarray(v_val, dtype=np.float32)
    input_pos = np.asarray(input_pos, dtype=np.int32)
    mask = np.asarray(mask, dtype=np.bool_)

    fast = input_pos.shape == (S,) and bool(
        (input_pos == np.arange(S, dtype=np.int32)).all())

    nc = _get_program(fast)
    in_maps = _shard_inputs(k_cache_q, k_scales, k_zero_points, v_cache_q,
                            v_scales, v_zero_points, k_val, v_val, mask)
    res = run_bass_kernel_spmd(nc, in_maps, core_ids=list(range(N_CORES)))
    k, v, m = _assemble(res.results)

    if not fast:
        # General-position fallback: apply the scatter on host.
        k[:, :, input_pos, :] = k_val
        v[:, :, input_pos, :] = v_val
        m[:, :, :, input_pos] = True

    return np.stack([k, v]), m
